# revision 51
# baseline (speedup 1.0000x reference)
"""Trainium2 Bass kernel for nn_Block_11321533792295 (dense transformer block).

Data-parallel over batch: 8 samples -> 8 NeuronCores, one sample each.
v2: phase 2 (attention finalize) is fused into the MLP sweep per 512-token
block, software-pipelined so the PE stays in a dense matmul stream (keeps
the HAM clock-gate warm and hides all cross-engine latency).  The residual
is added on the vector engine (no identity matmuls) and the out tensor
stays in SBUF (no DRAM round trip).
"""

import sys

sys.path.insert(0, "/opt/trn_rl_repo")

import numpy as np

import concourse.bacc as bacc
import concourse.bass as bass
import concourse.tile as tile
from concourse import mybir
from concourse.bass_utils import run_bass_kernel_spmd
from concourse.masks import make_identity

# Problem shapes (hardcoded per the harness contract).
B = 8
S = 4096
D = 768
I = 192
H = 3072
P = 128
EPS_LN = 1e-6

F32 = mybir.dt.float32
F32R = mybir.dt.float32r
BF16 = mybir.dt.bfloat16
F8E4 = mybir.dt.float8e4
DR = mybir.MatmulPerfMode.DoubleRow

# Mixed-precision MLP: first N_F8_UP d-chunks of the up-proj contraction and
# first N_F8_DN*128 hidden rows of the down-proj run as fp8 DoubleRow (2
# rows/cycle); the rest stays bf16.  w1 is pre-scaled x64 host-side (fp8
# subnormal floor), undone by the gelu input scale.
N_F8_UP = 2   # of 6 d-chunks (pairs -> 1 DoubleRow matmul)
N_F8_DN = 4   # of 24 hidden chunks (pairs -> 2 DoubleRow matmuls)
W1_SCALE = 64.0

N_TOK_TILES = S // P  # 32
N_BLK = 8  # blocks of 512 tokens
BLK = 512
N_DC = D // P  # 6 d-chunks
N_HC = H // P  # 24 hidden chunks
AF = mybir.ActivationFunctionType
ALU = mybir.AluOpType


def _phase1(nc, tc, const, dram, persist, mlpw):
    """LN1-mean + q/A + kT + per-token norms for all 32 token tiles."""
    (identity_b, ones_col, ones_two, eps_ln, zeros_f) = const
    x_d, qw_d, kw_d, wp_d, w1f8_d, w1_d, w2f8_d, w2_d = dram
    (qA_store, kT1_store, kT2_store, ssk_cols, rnk, G_row, rstdA,
     Gcol1, Gcol2, wp1_s, wp2_s, wf_s, mu2, rstd2) = persist
    w1f8_s, w1_s, w2f8_s, w2_s = mlpw

    with (
        tc.tile_pool(name="p1w", bufs=1) as p1w,
        tc.tile_pool(name="p1stat", bufs=8) as p1stat,
        tc.tile_pool(name="p1x", bufs=3) as p1x,
        tc.tile_pool(name="p1h", bufs=4) as p1h,
        tc.tile_pool(name="p1hT", bufs=2) as p1hT,
        tc.tile_pool(name="p1sq", bufs=2) as p1sq,
        tc.tile_pool(name="p1row", bufs=1) as p1row,
        tc.tile_pool(name="ps_t", bufs=2, space="PSUM") as ps_t,
        tc.tile_pool(name="ps_q", bufs=2, space="PSUM") as ps_q,
        tc.tile_pool(name="ps_k1", bufs=1, space="PSUM") as ps_k1,
        tc.tile_pool(name="ps_k2", bufs=1, space="PSUM") as ps_k2,
        tc.tile_pool(name="ps_small", bufs=1, space="PSUM") as ps_small,
        tc.tile_pool(name="ps_G", bufs=1, space="PSUM") as ps_G,
    ):
        qw_s = p1w.tile([P, N_DC, 256], BF16)
        kw_s = p1w.tile([P, N_DC, I], BF16)
        wp_s = p1w.tile([P, 2, I], BF16)
        nc.sync.dma_start(qw_s[:], qw_d.ap())
        nc.sync.dma_start(kw_s[:], kw_d.ap())
        nc.sync.dma_start(wp_s[:], wp_d.ap())

        nc.vector.tensor_copy(qA_store[:, :, 193], zeros_f[:])

        # x block loads with the MLP weight prefetch threaded between them on
        # the same (gpsimd) DMA ring, so phase-1's x never starves while the
        # big weights still land well before the fused phase needs them.
        xblocks = {}

        def load_x(b):
            # token map: partition p <- rows b*512 + 4p + t (contiguous 12KB
            # per partition -> large DMA packets).  Pure relabeling; every
            # per-token op downstream indexes (partition, slot) consistently
            # and y is written back through the same map.
            xb = p1x.tile([P, 4, D], F32R, tag="xblk", name=f"x_{b}")
            ring = nc.sync if b == 0 else nc.gpsimd
            ring.dma_start(
                xb[:],
                x_d.ap()[b * BLK : (b + 1) * BLK, :].rearrange(
                    "(p t) d -> p t d", p=P
                ),
            )
            xblocks[b] = xb

        def load_w1(piece):  # 4 pieces of 768 hidden cols each (pre-tiled host side)
            nc.gpsimd.dma_start(
                w1_s[:, :, piece * 768 : (piece + 1) * 768],
                w1_d.ap()[:, :, piece * 768 : (piece + 1) * 768],
            )

        def load_w2(piece):  # 2 pieces of 10 hidden chunks each (pre-tiled)
            nc.gpsimd.dma_start(
                w2_s[:, piece * 10 : (piece + 1) * 10, :],
                w2_d.ap()[:, piece * 10 : (piece + 1) * 10, :],
            )

        load_x(0)
        load_x(1)
        nc.gpsimd.dma_start(w1f8_s[:], w1f8_d.ap())
        load_w1(0)
        load_x(2)
        load_w1(1)
        load_x(3)
        load_w1(2)
        load_x(4)
        load_w1(3)
        load_x(5)
        nc.gpsimd.dma_start(w2f8_s[:], w2f8_d.ap())
        load_w2(0)
        load_x(6)
        load_w2(1)
        load_x(7)

        psum_G = ps_G.tile([1, 194], F32)
        ssk_rows = p1row.tile([1, N_BLK, BLK], F32R)
        hTs = {}

        def sweep1_tile(b, t4):
            # mean-center + transpose.  The per-token LN1 rstd cancels
            # exactly in the downstream l2norms (q-hat, k-hat, A-hat are
            # scale-invariant per token), so only the mean is needed.
            if t4 == 0:
                hTs[b] = p1hT.tile([P, N_DC, BLK], BF16, tag="hT", name=f"hT{b}")
            hT = hTs[b]
            xt = xblocks[b][:, t4, :]
            mu = p1stat.tile([P, 1], F32, tag="mu1")
            nc.vector.reduce_sum(mu[:], xt, axis=mybir.AxisListType.X)
            nc.vector.tensor_scalar_mul(mu[:], mu[:], 1.0 / D)
            h_nat = p1h.tile([P, D], BF16, tag="h_nat")
            nc.vector.tensor_scalar_sub(h_nat[:], xt, mu[:])
            pt = ps_t.tile([P, D], BF16, tag="ps_tr")
            for c in range(N_DC):
                nc.tensor.transpose(
                    pt[:, c * P : (c + 1) * P],
                    h_nat[:, c * P : (c + 1) * P],
                    identity_b[:],
                )
            # split the psum->sbuf copy between vector and scalar engines
            nc.vector.tensor_copy(
                hT[:, 0:3, t4 * P : (t4 + 1) * P],
                pt[:, 0 : 3 * P].rearrange("p (c n) -> p c n", c=3),
            )
            nc.scalar.copy(
                hT[:, 3:6, t4 * P : (t4 + 1) * P],
                pt[:, 3 * P : D].rearrange("p (c n) -> p c n", c=3),
            )
            if t4 == 3:
                xblocks.pop(b)

        def q_tile(b, t4):
            hT = hTs[b]
            t_glob = b * 4 + t4
            pq = ps_q.tile([P, 193], F32, tag="ps_q")
            for c in range(N_DC):
                nc.tensor.matmul(
                    pq[:],
                    hT[:, c, t4 * P : (t4 + 1) * P],
                    qw_s[:, c, 0:193],
                    start=(c == 0),
                    stop=(c == N_DC - 1),
                )
            sq_q = p1sq.tile([P, I], F32R, tag="sq_q")
            ssq = p1stat.tile([P, 1], F32, tag="ssq")
            nc.scalar.activation(
                sq_q[:], pq[:, 0:I], AF.Square, accum_out=ssq[:]
            )
            rnq = p1stat.tile([P, 1], F32, tag="rnq")
            nc.scalar.activation(rnq[:], ssq[:], AF.Sqrt)
            nc.vector.reciprocal(rnq[:], rnq[:])
            nc.scalar.activation(
                qA_store[:, t_glob, 0:193], pq[:, 0:193], AF.Copy,
                scale=rnq[:],
            )

        def k_block(b):
            hT = hTs[b]
            pk1 = ps_k1.tile([P, BLK], F32, tag="ps_k1")
            pk2 = ps_k2.tile([64, BLK], F32, tag="ps_k2")
            for c in range(N_DC):
                nc.tensor.matmul(
                    pk1[:],
                    kw_s[:, c, 0:P],
                    hT[:, c, :],
                    start=(c == 0),
                    stop=(c == N_DC - 1),
                )
            for c in range(N_DC):
                nc.tensor.matmul(
                    pk2[:],
                    kw_s[:, c, P:I],
                    hT[:, c, :],
                    start=(c == 0),
                    stop=(c == N_DC - 1),
                )
            nc.scalar.copy(kT1_store[:, b, :], pk1[:])
            nc.vector.tensor_copy(kT2_store[:, b, :], pk2[:])
            # sumsq_k row = ones.T @ (k^2), both chunks accumulated
            sqk1 = p1sq.tile([P, BLK], F32R, tag="sqk1")
            sqk2 = p1sq.tile([64, BLK], F32R, tag="sqk2")
            nc.scalar.activation(sqk1[:], pk1[:], AF.Square)
            nc.scalar.activation(sqk2[:], pk2[:], AF.Square)
            prow = ps_small.tile([1, BLK], F32, tag="ps_small", name="prow")
            nc.tensor.matmul(prow[:], ones_col[:], sqk1[:], start=True, stop=False)
            nc.tensor.matmul(
                prow[:], ones_col[:64, :], sqk2[:], start=False, stop=True
            )
            nc.vector.tensor_copy(ssk_rows[:, b, :], prow[:])
            # fold the row->column conversion into the block loop so the
            # post-loop tail stays short
            pcols = ps_small.tile([P, 8], F32, tag="ps_small", name=f"pcols{b}")
            for t4 in range(4):
                nc.tensor.matmul(
                    pcols[:, t4 * 2 : (t4 + 1) * 2],
                    ssk_rows[:, b, t4 * P : (t4 + 1) * P],
                    ones_two[:],
                    start=True,
                    stop=True,
                )
            nc.vector.tensor_copy(
                ssk_cols[:, b * 4 : b * 4 + 4],
                pcols[:].rearrange("p (t two) -> p t two", two=2)[:, :, 0],
            )
            # rnk for this block (keeps the post-loop tail short)
            nc.scalar.activation(
                rnk[:, b * 4 : b * 4 + 4], ssk_cols[:, b * 4 : b * 4 + 4],
                AF.Sqrt,
            )
            nc.vector.reciprocal(
                rnk[:, b * 4 : b * 4 + 4], rnk[:, b * 4 : b * 4 + 4]
            )
            hTs.pop(b)

        # software pipeline: block b+1's mean/transpose sweep rides between
        # block b's q tiles so the PE never drains.
        def g_chain(lo, hi):
            for t_glob in range(lo, hi):
                nc.tensor.matmul(
                    psum_G[:],
                    qA_store[:, t_glob, 192:193],
                    qA_store[:, t_glob, 0:194],
                    start=(t_glob == 0),
                    stop=(t_glob == N_TOK_TILES - 1),
                )

        for t4 in range(4):
            sweep1_tile(0, t4)
        for b in range(N_BLK):
            for t4 in range(4):
                q_tile(b, t4)
                if b + 1 < N_BLK:
                    sweep1_tile(b + 1, t4)
            k_block(b)
            if b == N_BLK - 2:
                # G partial sum over blocks 0..6 overlaps block 7's q tiles
                g_chain(0, (N_BLK - 1) * 4)
        g_chain((N_BLK - 1) * 4, N_TOK_TILES)
        # G finalisation
        nc.vector.tensor_copy(G_row[:], psum_G[:])
        sA = p1stat.tile([1, 1], F32, tag="sA")
        nc.scalar.activation(sA[:], G_row[:, 192:193].bitcast(F32), AF.Sqrt)
        nc.vector.reciprocal(sA[:], sA[:])
        nc.vector.tensor_copy(rstdA[:, 0:1], sA[:])
        nc.vector.tensor_copy(rstdA[:, 1:2], sA[:])
        pg1 = ps_small.tile([P, 2], F32, tag="ps_small", name="pg1")
        nc.tensor.matmul(pg1[:], G_row[:, 0:P], rstdA[:], start=True, stop=True)
        nc.vector.tensor_copy(Gcol1[:], pg1[:, 0:1])
        pg2 = ps_small.tile([64, 2], F32, tag="ps_small", name="pg2")
        nc.tensor.matmul(pg2[:], G_row[:, P:I], rstdA[:], start=True, stop=True)
        nc.vector.tensor_copy(Gcol2[:], pg2[:, 0:1])
        nc.vector.tensor_scalar_mul(wp1_s[:], wp_s[:, 0, :], Gcol1[:])
        nc.vector.tensor_scalar_mul(wp2_s[:], wp_s[:64, 1, :], Gcol2[:])


def _fused_phase(nc, tc, const, dram, persist, mlpw, pools):
    """Per-block: attention finalize + LN2 + MLP, software-pipelined."""
    (identity_b, ones_col, ones_two, eps_ln, zeros_f) = const
    x_d, y_d = dram
    (qA_store, kT1_store, kT2_store, ssk_cols, rnk, G_row, rstdA,
     Gcol1, Gcol2, wp1_s, wp2_s, wf_s, mu2, rstd2) = persist
    w1f8_s, w1_s, w2f8_s, w2_s = mlpw
    (xb_pool, foi, foiT, fout, fh2, fhT2, fg, ffin, fstat,
     ps_up, ps_dn, ps_tr, ps_oi) = pools

    state = {}

    def load_xres(b):
        xr = xb_pool.tile([P, 4, D], F32, tag="xres", name=f"xres{b}")
        nc.gpsimd.dma_start(
            xr[:],
            x_d.ap()[b * BLK : (b + 1) * BLK, :].rearrange("(p t) d -> p t d", p=P),
        )
        state[("xres", b)] = xr

    def attn_a(b):
        """out_inner matmuls + scale/add q-hat -> oi tiles (natural)."""
        ois = []
        for t4 in range(4):
            t_glob = b * 4 + t4
            poi = ps_oi.tile([P, I], F32, tag="ps_oi", name=f"poi{b}_{t4}")
            nc.tensor.matmul(
                poi[:],
                kT1_store[:, b, t4 * P : (t4 + 1) * P],
                wp1_s[:],
                start=True,
                stop=False,
            )
            nc.tensor.matmul(
                poi[:],
                kT2_store[:, b, t4 * P : (t4 + 1) * P],
                wp2_s[:],
                start=False,
                stop=True,
            )
            oi_t = foi.tile([P, I], F32R, tag="oi_t", bufs=2)
            nc.scalar.activation(
                oi_t[:], poi[:], AF.Copy, scale=rnk[:, t_glob : t_glob + 1]
            )
            oi = foi.tile([P, I], BF16, tag="oi", bufs=5)
            nc.vector.tensor_add(oi[:], oi_t[:], qA_store[:, t_glob, 0:I])
            ois.append(oi)
        state[("oi", b)] = ois

    def attn_t(b):
        """transpose oi tiles -> oiT1 [128, 512], oiT2 [64, 512]."""
        ois = state.pop(("oi", b))
        oiT1 = foiT.tile([P, BLK], BF16, tag="oiT1")
        oiT2 = foiT.tile([64, BLK], BF16, tag="oiT2")
        tp1 = ps_tr.tile([P, D], BF16, tag="ps_tr", name=f"ptoi1_{b}")
        tp2 = ps_tr.tile([P, D], BF16, tag="ps_tr", name=f"ptoi2_{b}")
        for t4 in range(4):
            oi = ois[t4]
            nc.tensor.transpose(
                tp1[:, t4 * P : (t4 + 1) * P], oi[:, 0:P], identity_b[:]
            )
            nc.tensor.transpose(
                tp2[0:64, t4 * P : (t4 + 1) * P], oi[:, P:I], identity_b[:]
            )
        nc.vector.tensor_copy(oiT1[:], tp1[:, 0:BLK])
        nc.scalar.copy(oiT2[:], tp2[0:64, 0:BLK])
        state[("oiT", b)] = (oiT1, oiT2)

    def attn_b(b):
        """final attn matmuls + residual add (DVE) + LN2 stats."""
        oiT1, oiT2 = state.pop(("oiT", b))
        xr = state.pop(("xres", b))
        outb = fout.tile([P, 4, D], BF16, tag="outb")
        # all residual adds first so the down-proj psum ring drains promptly
        for t4 in range(4):
            for nh in range(2):
                pf = ps_dn.tile([P, 384], F32, tag="ps_dn", name=f"pf{b}_{t4}_{nh}")
                nc.tensor.matmul(
                    pf[:],
                    oiT1[:, t4 * P : (t4 + 1) * P],
                    wf_s[:, 0, nh * 384 : (nh + 1) * 384],
                    start=True,
                    stop=False,
                )
                nc.tensor.matmul(
                    pf[:],
                    oiT2[:, t4 * P : (t4 + 1) * P],
                    wf_s[:64, 1, nh * 384 : (nh + 1) * 384],
                    start=False,
                    stop=True,
                )
                nc.vector.tensor_add(
                    outb[:, t4, nh * 384 : (nh + 1) * 384],
                    pf[:],
                    xr[:, t4, nh * 384 : (nh + 1) * 384],
                )
        for t4 in range(4):
            t_glob = b * 4 + t4
            stats = fstat.tile([P, 3, 6], F32, tag="bn_stats")
            for sg in range(3):
                nc.vector.bn_stats(
                    stats[:, sg, :], outb[:, t4, sg * 256 : (sg + 1) * 256]
                )
            mv = fstat.tile([P, 2], F32, tag="bn_mv")
            nc.vector.bn_aggr(mv[:], stats[:])
            nc.vector.tensor_copy(mu2[:, t_glob : t_glob + 1], mv[:, 0:1])
            nc.vector.tensor_copy(rstd2[:, t_glob : t_glob + 1], mv[:, 1:2])
        # rstd for this block's 4 tiles
        nc.scalar.activation(
            rstd2[:, b * 4 : b * 4 + 4],
            rstd2[:, b * 4 : b * 4 + 4],
            AF.Sqrt,
            bias=eps_ln[:],
        )
        nc.vector.reciprocal(rstd2[:, b * 4 : b * 4 + 4], rstd2[:, b * 4 : b * 4 + 4])
        state[("out", b)] = outb

    def attn_c(b):
        """LN2 apply + transpose -> hT2 fp8 chunks 0:2 + bf16 chunks 2:6."""
        outb = state[("out", b)]
        hT2f = fhT2.tile([P, N_F8_UP, BLK], F8E4, tag="hT2f", name=f"hT2f_{b}")
        hT2 = fhT2.tile([P, N_DC - N_F8_UP, BLK], BF16, tag="hT2",
                        name=f"hT2_{b}")
        for tt in range(4):
            t_glob = b * 4 + tt
            h2 = fh2.tile([P, D], BF16, tag="h2", name=f"h2_{b}_{tt}")
            nc.vector.tensor_scalar(
                out=h2[:],
                in0=outb[:, tt, :],
                scalar1=mu2[:, t_glob : t_glob + 1],
                scalar2=rstd2[:, t_glob : t_glob + 1],
                op0=ALU.subtract,
                op1=ALU.mult,
            )
            pt = ps_tr.tile([P, D], BF16, tag="ps_tr", name=f"pt3_{b}_{tt}")
            for c in range(N_DC):
                nc.tensor.transpose(
                    pt[:, c * P : (c + 1) * P],
                    h2[:, c * P : (c + 1) * P],
                    identity_b[:],
                )
            nc.vector.tensor_copy(
                hT2f[:, :, tt * P : (tt + 1) * P],
                pt[:, 0 : N_F8_UP * P].rearrange("p (c n) -> p c n", c=N_F8_UP),
            )
            nc.scalar.copy(
                hT2[:, 0:2, tt * P : (tt + 1) * P],
                pt[:, N_F8_UP * P : 4 * P].rearrange("p (c n) -> p c n", c=2),
            )
            nc.scalar.copy(
                hT2[:, 2:4, tt * P : (tt + 1) * P],
                pt[:, 4 * P : D].rearrange("p (c n) -> p c n", c=2),
            )
        state[("hT2", b)] = (hT2f, hT2)

    def mlp_up(b, j_lo, j_hi):
        if j_lo == 0:
            state[("g", b)] = (
                fg.tile([P, N_F8_DN, BLK], F8E4, tag="gf8", name=f"gf8_{b}"),
                fg.tile([P, N_HC - N_F8_DN, BLK], BF16, tag="gb",
                        name=f"gb_{b}"),
            )
        gf8, gb = state[("g", b)]
        hT2f, hT2 = state[("hT2", b)]
        for j in range(j_lo, j_hi):
            pu = ps_up.tile([P, BLK], F32, tag="ps_up")
            nc.tensor.matmul(
                pu[:],
                w1f8_s[:, :, j * P : (j + 1) * P],
                hT2f[:],
                start=True,
                stop=False,
                perf_mode=DR,
            )
            for c in range(N_DC - N_F8_UP):
                nc.tensor.matmul(
                    pu[:],
                    w1_s[:, c, j * P : (j + 1) * P],
                    hT2[:, c, :],
                    start=False,
                    stop=(c == N_DC - N_F8_UP - 1),
                )
            if j < N_F8_DN:
                nc.scalar.activation(
                    gf8[:, j, :], pu[:], AF.Gelu, scale=1.0 / W1_SCALE
                )
            else:
                nc.scalar.activation(
                    gb[:, j - N_F8_DN, :], pu[:], AF.Gelu, scale=1.0 / W1_SCALE
                )
        if j_hi == N_HC:
            state.pop(("hT2", b))

    def mlp_dn(b, tt_lo, tt_hi):
        gf8, gb = state[("g", b)]
        outb = state[("out", b)]
        finb = ffin.tile([P, 2, D], F32, tag="finb", name=f"finb{b}_{tt_lo}")
        for tt in range(tt_lo, tt_hi):
            for nh in range(2):
                py = ps_dn.tile([P, 384], F32, tag="ps_dn", name=f"py{b}_{tt}_{nh}")
                for m in range(N_F8_DN // 2):
                    nc.tensor.matmul(
                        py[:],
                        gf8[:, 2 * m : 2 * m + 2, tt * P : (tt + 1) * P],
                        w2f8_s[:, 2 * m : 2 * m + 2, nh * 384 : (nh + 1) * 384],
                        start=(m == 0),
                        stop=False,
                        perf_mode=DR,
                    )
                for j in range(N_HC - N_F8_DN):
                    nc.tensor.matmul(
                        py[:],
                        gb[:, j, tt * P : (tt + 1) * P],
                        w2_s[:, j, nh * 384 : (nh + 1) * 384],
                        start=False,
                        stop=(j == N_HC - N_F8_DN - 1),
                    )
                nc.vector.tensor_add(
                    finb[:, tt - tt_lo, nh * 384 : (nh + 1) * 384],
                    py[:],
                    outb[:, tt, nh * 384 : (nh + 1) * 384],
                )
            if b == N_BLK - 1:
                # split the final writes so the kernel-end drain is short
                nc.sync.dma_start(
                    y_d.ap()[b * BLK : (b + 1) * BLK, :].rearrange(
                        "(p t) d -> p t d", p=P
                    )[:, tt : tt + 1, :],
                    finb[:, tt - tt_lo : tt - tt_lo + 1, :],
                )
        if b != N_BLK - 1:
            nc.sync.dma_start(
                y_d.ap()[b * BLK : (b + 1) * BLK, :].rearrange(
                    "(p t) d -> p t d", p=P
                )[:, tt_lo:tt_hi, :],
                finb[:],
            )
        if tt_hi == 4:
            state.pop(("g", b))
            state.pop(("out", b))

    # prologue: full attention chain for block 0
    load_xres(0)
    load_xres(1)
    attn_a(0)
    attn_t(0)
    attn_b(0)
    attn_c(0)
    for b in range(N_BLK):
        if b + 2 < N_BLK:
            load_xres(b + 2)
        if b + 1 < N_BLK:
            attn_a(b + 1)
        mlp_up(b, 0, 12)
        if b + 1 < N_BLK:
            attn_t(b + 1)
        mlp_up(b, 12, N_HC)
        if b + 1 < N_BLK:
            attn_b(b + 1)
        mlp_dn(b, 0, 2)
        if b + 1 < N_BLK:
            attn_c(b + 1)
        mlp_dn(b, 2, 4)


def build_nc():
    nc = bacc.Bacc(trn_type="TRN2")

    # Per-core inputs (weights replicated across cores, x sliced per core).
    # Weights arrive pre-tiled host-side into their exact SBUF layouts so
    # every weight DMA is a contiguous per-partition copy (large packets).
    x_d = nc.dram_tensor("x", [S, D], F32R, kind="ExternalInput")
    qw_d = nc.dram_tensor("qw", [P, N_DC, 256], BF16, kind="ExternalInput")
    kw_d = nc.dram_tensor("kw", [P, N_DC, I], BF16, kind="ExternalInput")
    wp_d = nc.dram_tensor("wp", [P, 2, I], BF16, kind="ExternalInput")
    wf_d = nc.dram_tensor("wf", [P, 2, D], BF16, kind="ExternalInput")
    w1f8_d = nc.dram_tensor("w1f8", [P, N_F8_UP, H], F8E4, kind="ExternalInput")
    w1_d = nc.dram_tensor("w1", [P, N_DC - N_F8_UP, H], BF16, kind="ExternalInput")
    w2f8_d = nc.dram_tensor("w2f8", [P, N_F8_DN, D], F8E4, kind="ExternalInput")
    w2_d = nc.dram_tensor("w2", [P, N_HC - N_F8_DN, D], BF16, kind="ExternalInput")
    y_d = nc.dram_tensor("y", [S, D], F32, kind="ExternalOutput")

    with tile.TileContext(nc) as tc:
        with (
            tc.tile_pool(name="const", bufs=1) as const_pool,
            tc.tile_pool(name="p3w", bufs=1) as p3w,
            tc.tile_pool(name="persist", bufs=1) as pers,
        ):
            identity_f = const_pool.tile([P, P], F32)
            make_identity(nc, identity_f[:])
            identity_b = const_pool.tile([P, P], BF16)
            nc.vector.tensor_copy(identity_b[:], identity_f[:])
            ones_f = const_pool.tile([P, 2], F32)
            nc.vector.memset(ones_f[:], 1.0)
            ones_col = const_pool.tile([P, 1], F32R)
            nc.vector.tensor_copy(ones_col[:], ones_f[:, 0:1])
            ones_two = const_pool.tile([1, 2], F32R)
            nc.vector.tensor_copy(ones_two[:], ones_f[0:1, :])
            eps_ln = const_pool.tile([P, 1], F32)
            nc.vector.memset(eps_ln[:], EPS_LN)
            zeros_f = const_pool.tile([P, N_TOK_TILES], F32)
            nc.vector.memset(zeros_f[:], 0.0)
            const = (identity_b, ones_col, ones_two, eps_ln, zeros_f)

            # Persistent per-sample state (phase 1 -> fused phase).
            qA_store = pers.tile([P, N_TOK_TILES, 194], BF16)
            kT1_store = pers.tile([P, N_BLK, BLK], BF16)
            kT2_store = pers.tile([64, N_BLK, BLK], BF16)
            ssk_cols = pers.tile([P, N_TOK_TILES], F32)
            rnk = pers.tile([P, N_TOK_TILES], F32)
            G_row = pers.tile([1, 194], F32R)
            rstdA = pers.tile([1, 2], F32R)
            Gcol1 = pers.tile([P, 1], F32)
            Gcol2 = pers.tile([64, 1], F32)
            wp1_s = pers.tile([P, I], BF16)
            wp2_s = pers.tile([64, I], BF16)
            wf_s = pers.tile([P, 2, D], BF16)
            nc.sync.dma_start(wf_s[:], wf_d.ap())
            mu2 = pers.tile([P, N_TOK_TILES], F32)
            rstd2 = pers.tile([P, N_TOK_TILES], F32)
            persist = (qA_store, kT1_store, kT2_store, ssk_cols, rnk, G_row,
                       rstdA, Gcol1, Gcol2, wp1_s, wp2_s, wf_s,
                       mu2, rstd2)

            w1f8_s = p3w.tile([P, N_F8_UP, H], F8E4)
            w1_s = p3w.tile([P, N_DC - N_F8_UP, H], BF16)
            w2f8_s = p3w.tile([P, N_F8_DN, D], F8E4)
            w2_s = p3w.tile([P, N_HC - N_F8_DN, D], BF16)
            mlpw = (w1f8_s, w1_s, w2f8_s, w2_s)

            _phase1(nc, tc, const,
                    (x_d, qw_d, kw_d, wp_d, w1f8_d, w1_d, w2f8_d, w2_d),
                    persist, mlpw)

            with (
                tc.tile_pool(name="xres", bufs=2) as xb_pool,
                tc.tile_pool(name="foi", bufs=4) as foi,
                tc.tile_pool(name="foiT", bufs=2) as foiT,
                tc.tile_pool(name="fout", bufs=2) as fout,
                tc.tile_pool(name="fh2", bufs=2) as fh2,
                tc.tile_pool(name="fhT2", bufs=2) as fhT2,
                tc.tile_pool(name="fg", bufs=1) as fg,
                tc.tile_pool(name="ffin", bufs=2) as ffin,
                tc.tile_pool(name="fstat", bufs=2) as fstat,
                tc.tile_pool(name="ps_up", bufs=2, space="PSUM") as ps_up,
                tc.tile_pool(name="ps_dn", bufs=2, space="PSUM") as ps_dn,
                tc.tile_pool(name="ps_tr2", bufs=2, space="PSUM") as ps_tr2,
                tc.tile_pool(name="ps_oi", bufs=2, space="PSUM") as ps_oi,
            ):
                pools = (xb_pool, foi, foiT, fout, fh2, fhT2, fg, ffin, fstat,
                         ps_up, ps_dn, ps_tr2, ps_oi)
                _fused_phase(nc, tc, const, (x_d, y_d), persist, mlpw, pools)

    nc.finalize()
    return nc


_NC_CACHE = {}


def _get_nc():
    if "nc" not in _NC_CACHE:
        _NC_CACHE["nc"] = build_nc()
    return _NC_CACHE["nc"]


def kernel(
    x,
    ln1_g,
    ln1_b,
    wq,
    bq,
    wk,
    bk,
    w_g,
    w_proj,
    b_proj,
    w_final,
    b_final,
    ln2_g,
    ln2_b,
    w1,
    b1,
    w2,
    b2,
    _trace=False,
    _trace_kwargs=None,
):
    import ml_dtypes

    x = np.asarray(x, dtype=np.float32)
    f = lambda a: np.asarray(a, dtype=np.float32)
    ln1_g, ln1_b, ln2_g, ln2_b = f(ln1_g), f(ln1_b), f(ln2_g), f(ln2_b)
    wq, bq, wk, bk = f(wq), f(bq), f(wk), f(bk)
    w_g, w_proj, b_proj = f(w_g), f(w_proj), f(b_proj)
    w_final, b_final, w1, b1, w2, b2 = f(w_final), f(b_final), f(w1), f(b1), f(w2), f(b2)

    # The kernel folds LN gains into the weights and relies on all additive
    # biases being zero (guaranteed by the problem's setup_inputs).
    for name, bias in [
        ("ln1_b", ln1_b),
        ("bq", bq),
        ("bk", bk),
        ("b_proj", b_proj),
        ("b_final", b_final),
        ("ln2_b", ln2_b),
        ("b1", b1),
        ("b2", b2),
    ]:
        assert not np.any(bias), f"kernel assumes {name} == 0"

    wq_eff = ln1_g[:, None] * wq  # [768, 192]
    wk_eff = ln1_g[:, None] * wk
    wq_g = wq_eff @ w_g  # [768, 1]
    qw_host = np.concatenate(
        [wq_eff, wq_g, np.zeros((D, 63), np.float32)], axis=1
    ).astype(np.float32)
    w1_eff = (ln2_g[:, None] * w1).astype(ml_dtypes.bfloat16)

    bf = ml_dtypes.bfloat16

    def tile_rows(a, n_chunks):  # [n*128, m] -> [128, n, m]
        return np.ascontiguousarray(
            a.reshape(n_chunks, P, a.shape[1]).transpose(1, 0, 2)
        )

    def split192(a):  # [192, m] -> [128, 2, m] (second slot half-filled)
        out = np.zeros((P, 2, a.shape[1]), dtype=a.dtype)
        out[:, 0, :] = a[0:P]
        out[0:64, 1, :] = a[P:I]
        return out

    f8 = ml_dtypes.float8_e4m3
    w1_scaled = (ln2_g[:, None] * w1) * W1_SCALE  # f32
    cut_up = N_F8_UP * P
    cut_dn = N_F8_DN * P
    nc = _get_nc()
    weights = {
        "qw": tile_rows(qw_host.astype(bf), N_DC),
        "kw": tile_rows(wk_eff.astype(bf), N_DC),
        "wp": split192(w_proj.astype(bf)),
        "wf": split192(w_final.astype(bf)),
        "w1f8": tile_rows(w1_scaled[:cut_up].astype(f8), N_F8_UP),
        "w1": tile_rows(w1_scaled[cut_up:].astype(bf), N_DC - N_F8_UP),
        "w2f8": tile_rows(w2[:cut_dn].astype(f8), N_F8_DN),
        "w2": tile_rows(w2[cut_dn:].astype(bf), N_HC - N_F8_DN),
    }
    in_maps = [dict(weights, x=np.ascontiguousarray(x[i])) for i in range(B)]
    # The first execution after a fresh NEFF load occasionally trips a
    # transient NRT_EXEC_UNIT_UNRECOVERABLE; a retry has always succeeded.
    last_err = None
    for attempt in range(3):
        try:
            res = run_bass_kernel_spmd(
                nc,
                in_maps,
                core_ids=list(range(B)),
                trace=_trace,
                **(_trace_kwargs or {}),
            )
            break
        except Exception as e:  # noqa: BLE001
            last_err = e
            if attempt == 2:
                raise
    else:
        raise last_err
    out = np.stack([res.results[i]["y"] for i in range(B)], axis=0)
    if _trace:
        return out, res
    return out


if __name__ == "__main__":
    print("building...")
    nc = _get_nc()
    print("built")


# revision 52
# speedup vs baseline: 1.0264x; 1.0264x over previous
"""Trainium2 Bass kernel for nn_Block_11321533792295 (dense transformer block).

Data-parallel over batch: 8 samples -> 8 NeuronCores, one sample each.
v2: phase 2 (attention finalize) is fused into the MLP sweep per 512-token
block, software-pipelined so the PE stays in a dense matmul stream (keeps
the HAM clock-gate warm and hides all cross-engine latency).  The residual
is added on the vector engine (no identity matmuls) and the out tensor
stays in SBUF (no DRAM round trip).
"""

import sys

sys.path.insert(0, "/opt/trn_rl_repo")

import numpy as np

import concourse.bacc as bacc
import concourse.bass as bass
import concourse.tile as tile
from concourse import mybir
from concourse.bass_utils import run_bass_kernel_spmd
from concourse.masks import make_identity

# Problem shapes (hardcoded per the harness contract).
B = 8
S = 4096
D = 768
I = 192
H = 3072
P = 128
EPS_LN = 1e-6

F32 = mybir.dt.float32
F32R = mybir.dt.float32r
BF16 = mybir.dt.bfloat16
F8E4 = mybir.dt.float8e4
DR = mybir.MatmulPerfMode.DoubleRow

# Mixed-precision MLP: first N_F8_UP d-chunks of the up-proj contraction and
# first N_F8_DN*128 hidden rows of the down-proj run as fp8 DoubleRow (2
# rows/cycle); the rest stays bf16.  w1 is pre-scaled x64 host-side (fp8
# subnormal floor), undone by the gelu input scale.
N_F8_UP = 2   # of 6 d-chunks (pairs -> 1 DoubleRow matmul)
N_F8_DN = 4   # of 24 hidden chunks (pairs -> 2 DoubleRow matmuls)
W1_SCALE = 64.0

N_TOK_TILES = S // P  # 32
N_BLK = 8  # blocks of 512 tokens
BLK = 512
N_DC = D // P  # 6 d-chunks
N_HC = H // P  # 24 hidden chunks
AF = mybir.ActivationFunctionType
ALU = mybir.AluOpType


def _phase1(nc, tc, const, dram, persist, mlpw):
    """LN1-mean + q/A + kT + per-token norms for all 32 token tiles."""
    (identity_b, ones_col, ones_two, eps_ln, zeros_f) = const
    x_d, qw_d, kw_d, wp_d, w1f8_d, w1_d, w2f8_d, w2_d = dram
    (qA_store, kT1_store, kT2_store, ssk_cols, rnk, G_row, rstdA,
     Gcol1, Gcol2, wp1_s, wp2_s, wf_s, mu2, rstd2) = persist
    w1f8_s, w1_s, w2f8_s, w2_s = mlpw

    with (
        tc.tile_pool(name="p1w", bufs=1) as p1w,
        tc.tile_pool(name="p1stat", bufs=8) as p1stat,
        tc.tile_pool(name="p1x", bufs=3) as p1x,
        tc.tile_pool(name="p1h", bufs=4) as p1h,
        tc.tile_pool(name="p1hT", bufs=2) as p1hT,
        tc.tile_pool(name="p1sq", bufs=2) as p1sq,
        tc.tile_pool(name="p1row", bufs=1) as p1row,
        tc.tile_pool(name="ps_t", bufs=2, space="PSUM") as ps_t,
        tc.tile_pool(name="ps_q", bufs=2, space="PSUM") as ps_q,
        tc.tile_pool(name="ps_k1", bufs=1, space="PSUM") as ps_k1,
        tc.tile_pool(name="ps_k2", bufs=1, space="PSUM") as ps_k2,
        tc.tile_pool(name="ps_small", bufs=1, space="PSUM") as ps_small,
        tc.tile_pool(name="ps_G", bufs=1, space="PSUM") as ps_G,
    ):
        qw_s = p1w.tile([P, N_DC, 256], BF16)
        kw_s = p1w.tile([P, N_DC, I], BF16)
        wp_s = p1w.tile([P, 2, I], BF16)
        nc.sync.dma_start(qw_s[:], qw_d.ap())
        nc.sync.dma_start(kw_s[:], kw_d.ap())
        nc.sync.dma_start(wp_s[:], wp_d.ap())

        nc.vector.tensor_copy(qA_store[:, :, 193], zeros_f[:])

        # x block loads with the MLP weight prefetch threaded between them on
        # the same (gpsimd) DMA ring, so phase-1's x never starves while the
        # big weights still land well before the fused phase needs them.
        xblocks = {}

        def load_x(b):
            # token map: partition p <- rows b*512 + 4p + t (contiguous 12KB
            # per partition -> large DMA packets).  Pure relabeling; every
            # per-token op downstream indexes (partition, slot) consistently
            # and y is written back through the same map.
            xb = p1x.tile([P, 4, D], F32R, tag="xblk", name=f"x_{b}")
            ring = nc.gpsimd
            ring.dma_start(
                xb[:],
                x_d.ap()[b * BLK : (b + 1) * BLK, :].rearrange(
                    "(p t) d -> p t d", p=P
                ),
            )
            xblocks[b] = xb

        def load_w1(piece):  # 4 pieces of 768 hidden cols each (pre-tiled host side)
            nc.gpsimd.dma_start(
                w1_s[:, :, piece * 768 : (piece + 1) * 768],
                w1_d.ap()[:, :, piece * 768 : (piece + 1) * 768],
            )

        def load_w2(piece):  # 2 pieces of 10 hidden chunks each (pre-tiled)
            nc.gpsimd.dma_start(
                w2_s[:, piece * 10 : (piece + 1) * 10, :],
                w2_d.ap()[:, piece * 10 : (piece + 1) * 10, :],
            )

        load_x(0)
        load_x(1)
        nc.gpsimd.dma_start(w1f8_s[:], w1f8_d.ap())
        load_w1(0)
        load_x(2)
        load_w1(1)
        load_x(3)
        load_w1(2)
        load_x(4)
        load_w1(3)
        load_x(5)
        nc.gpsimd.dma_start(w2f8_s[:], w2f8_d.ap())
        load_w2(0)
        load_x(6)
        load_w2(1)
        load_x(7)

        psum_G = ps_G.tile([1, 194], F32)
        ssk_rows = p1row.tile([1, N_BLK, BLK], F32R)
        hTs = {}

        def sweep1_tile(b, t4):
            # mean-center + transpose.  The per-token LN1 rstd cancels
            # exactly in the downstream l2norms (q-hat, k-hat, A-hat are
            # scale-invariant per token), so only the mean is needed.
            if t4 == 0:
                hTs[b] = p1hT.tile([P, N_DC, BLK], BF16, tag="hT", name=f"hT{b}")
            hT = hTs[b]
            xt = xblocks[b][:, t4, :]
            mu = p1stat.tile([P, 1], F32, tag="mu1")
            nc.vector.reduce_sum(mu[:], xt, axis=mybir.AxisListType.X)
            nc.vector.tensor_scalar_mul(mu[:], mu[:], 1.0 / D)
            h_nat = p1h.tile([P, D], BF16, tag="h_nat")
            nc.vector.tensor_scalar_sub(h_nat[:], xt, mu[:])
            pt = ps_t.tile([P, D], BF16, tag="ps_tr")
            for c in range(N_DC):
                nc.tensor.transpose(
                    pt[:, c * P : (c + 1) * P],
                    h_nat[:, c * P : (c + 1) * P],
                    identity_b[:],
                )
            # split the psum->sbuf copy between vector and scalar engines
            nc.vector.tensor_copy(
                hT[:, 0:3, t4 * P : (t4 + 1) * P],
                pt[:, 0 : 3 * P].rearrange("p (c n) -> p c n", c=3),
            )
            nc.scalar.copy(
                hT[:, 3:6, t4 * P : (t4 + 1) * P],
                pt[:, 3 * P : D].rearrange("p (c n) -> p c n", c=3),
            )
            if t4 == 3:
                xblocks.pop(b)

        def q_tile(b, t4):
            hT = hTs[b]
            t_glob = b * 4 + t4
            pq = ps_q.tile([P, 193], F32, tag="ps_q")
            for c in range(N_DC):
                nc.tensor.matmul(
                    pq[:],
                    hT[:, c, t4 * P : (t4 + 1) * P],
                    qw_s[:, c, 0:193],
                    start=(c == 0),
                    stop=(c == N_DC - 1),
                )
            sq_q = p1sq.tile([P, I], F32R, tag="sq_q")
            ssq = p1stat.tile([P, 1], F32, tag="ssq")
            nc.scalar.activation(
                sq_q[:], pq[:, 0:I], AF.Square, accum_out=ssq[:]
            )
            rnq = p1stat.tile([P, 1], F32, tag="rnq")
            nc.scalar.activation(rnq[:], ssq[:], AF.Sqrt)
            nc.vector.reciprocal(rnq[:], rnq[:])
            nc.scalar.activation(
                qA_store[:, t_glob, 0:193], pq[:, 0:193], AF.Copy,
                scale=rnq[:],
            )

        def k_block(b):
            hT = hTs[b]
            pk1 = ps_k1.tile([P, BLK], F32, tag="ps_k1")
            pk2 = ps_k2.tile([64, BLK], F32, tag="ps_k2")
            for c in range(N_DC):
                nc.tensor.matmul(
                    pk1[:],
                    kw_s[:, c, 0:P],
                    hT[:, c, :],
                    start=(c == 0),
                    stop=(c == N_DC - 1),
                )
            for c in range(N_DC):
                nc.tensor.matmul(
                    pk2[:],
                    kw_s[:, c, P:I],
                    hT[:, c, :],
                    start=(c == 0),
                    stop=(c == N_DC - 1),
                )
            nc.scalar.copy(kT1_store[:, b, :], pk1[:])
            nc.vector.tensor_copy(kT2_store[:, b, :], pk2[:])
            # sumsq_k row = ones.T @ (k^2), both chunks accumulated
            sqk1 = p1sq.tile([P, BLK], F32R, tag="sqk1")
            sqk2 = p1sq.tile([64, BLK], F32R, tag="sqk2")
            nc.scalar.activation(sqk1[:], pk1[:], AF.Square)
            nc.scalar.activation(sqk2[:], pk2[:], AF.Square)
            prow = ps_small.tile([1, BLK], F32, tag="ps_small", name="prow")
            nc.tensor.matmul(prow[:], ones_col[:], sqk1[:], start=True, stop=False)
            nc.tensor.matmul(
                prow[:], ones_col[:64, :], sqk2[:], start=False, stop=True
            )
            nc.vector.tensor_copy(ssk_rows[:, b, :], prow[:])
            # fold the row->column conversion into the block loop so the
            # post-loop tail stays short
            pcols = ps_small.tile([P, 8], F32, tag="ps_small", name=f"pcols{b}")
            for t4 in range(4):
                nc.tensor.matmul(
                    pcols[:, t4 * 2 : (t4 + 1) * 2],
                    ssk_rows[:, b, t4 * P : (t4 + 1) * P],
                    ones_two[:],
                    start=True,
                    stop=True,
                )
            nc.vector.tensor_copy(
                ssk_cols[:, b * 4 : b * 4 + 4],
                pcols[:].rearrange("p (t two) -> p t two", two=2)[:, :, 0],
            )
            # rnk for this block (keeps the post-loop tail short)
            nc.scalar.activation(
                rnk[:, b * 4 : b * 4 + 4], ssk_cols[:, b * 4 : b * 4 + 4],
                AF.Sqrt,
            )
            nc.vector.reciprocal(
                rnk[:, b * 4 : b * 4 + 4], rnk[:, b * 4 : b * 4 + 4]
            )
            hTs.pop(b)

        # software pipeline: block b+1's mean/transpose sweep rides between
        # block b's q tiles so the PE never drains.
        def g_chain(lo, hi):
            for t_glob in range(lo, hi):
                nc.tensor.matmul(
                    psum_G[:],
                    qA_store[:, t_glob, 192:193],
                    qA_store[:, t_glob, 0:194],
                    start=(t_glob == 0),
                    stop=(t_glob == N_TOK_TILES - 1),
                )

        for t4 in range(4):
            sweep1_tile(0, t4)
        for b in range(N_BLK):
            for t4 in range(4):
                q_tile(b, t4)
                if b + 1 < N_BLK:
                    sweep1_tile(b + 1, t4)
            k_block(b)
            if b == N_BLK - 2:
                # G partial sum over blocks 0..6 overlaps block 7's q tiles
                g_chain(0, (N_BLK - 1) * 4)
        g_chain((N_BLK - 1) * 4, N_TOK_TILES)
        # G finalisation
        nc.vector.tensor_copy(G_row[:], psum_G[:])
        sA = p1stat.tile([1, 1], F32, tag="sA")
        nc.scalar.activation(sA[:], G_row[:, 192:193].bitcast(F32), AF.Sqrt)
        nc.vector.reciprocal(sA[:], sA[:])
        nc.vector.tensor_copy(rstdA[:, 0:1], sA[:])
        nc.vector.tensor_copy(rstdA[:, 1:2], sA[:])
        pg1 = ps_small.tile([P, 2], F32, tag="ps_small", name="pg1")
        nc.tensor.matmul(pg1[:], G_row[:, 0:P], rstdA[:], start=True, stop=True)
        nc.vector.tensor_copy(Gcol1[:], pg1[:, 0:1])
        pg2 = ps_small.tile([64, 2], F32, tag="ps_small", name="pg2")
        nc.tensor.matmul(pg2[:], G_row[:, P:I], rstdA[:], start=True, stop=True)
        nc.vector.tensor_copy(Gcol2[:], pg2[:, 0:1])
        nc.vector.tensor_scalar_mul(wp1_s[:], wp_s[:, 0, :], Gcol1[:])
        nc.vector.tensor_scalar_mul(wp2_s[:], wp_s[:64, 1, :], Gcol2[:])


def _fused_phase(nc, tc, const, dram, persist, mlpw, pools):
    """Per-block: attention finalize + LN2 + MLP, software-pipelined."""
    (identity_b, ones_col, ones_two, eps_ln, zeros_f) = const
    x_d, y_d = dram
    (qA_store, kT1_store, kT2_store, ssk_cols, rnk, G_row, rstdA,
     Gcol1, Gcol2, wp1_s, wp2_s, wf_s, mu2, rstd2) = persist
    w1f8_s, w1_s, w2f8_s, w2_s = mlpw
    (xb_pool, foi, foiT, fout, fh2, fhT2, fg, ffin, fstat,
     ps_up, ps_dn, ps_tr, ps_oi) = pools

    state = {}

    def load_xres(b):
        xr = xb_pool.tile([P, 4, D], F32, tag="xres", name=f"xres{b}")
        nc.gpsimd.dma_start(
            xr[:],
            x_d.ap()[b * BLK : (b + 1) * BLK, :].rearrange("(p t) d -> p t d", p=P),
        )
        state[("xres", b)] = xr

    def attn_a(b):
        """out_inner matmuls + scale/add q-hat -> oi tiles (natural)."""
        ois = []
        for t4 in range(4):
            t_glob = b * 4 + t4
            poi = ps_oi.tile([P, I], F32, tag="ps_oi", name=f"poi{b}_{t4}")
            nc.tensor.matmul(
                poi[:],
                kT1_store[:, b, t4 * P : (t4 + 1) * P],
                wp1_s[:],
                start=True,
                stop=False,
            )
            nc.tensor.matmul(
                poi[:],
                kT2_store[:, b, t4 * P : (t4 + 1) * P],
                wp2_s[:],
                start=False,
                stop=True,
            )
            oi_t = foi.tile([P, I], F32R, tag="oi_t", bufs=2)
            nc.scalar.activation(
                oi_t[:], poi[:], AF.Copy, scale=rnk[:, t_glob : t_glob + 1]
            )
            oi = foi.tile([P, I], BF16, tag="oi", bufs=5)
            nc.vector.tensor_add(oi[:], oi_t[:], qA_store[:, t_glob, 0:I])
            ois.append(oi)
        state[("oi", b)] = ois

    def attn_t(b):
        """transpose oi tiles -> oiT1 [128, 512], oiT2 [64, 512]."""
        ois = state.pop(("oi", b))
        oiT1 = foiT.tile([P, BLK], BF16, tag="oiT1")
        oiT2 = foiT.tile([64, BLK], BF16, tag="oiT2")
        tp1 = ps_tr.tile([P, D], BF16, tag="ps_tr", name=f"ptoi1_{b}")
        tp2 = ps_tr.tile([P, D], BF16, tag="ps_tr", name=f"ptoi2_{b}")
        for t4 in range(4):
            oi = ois[t4]
            nc.tensor.transpose(
                tp1[:, t4 * P : (t4 + 1) * P], oi[:, 0:P], identity_b[:]
            )
            nc.tensor.transpose(
                tp2[0:64, t4 * P : (t4 + 1) * P], oi[:, P:I], identity_b[:]
            )
        nc.vector.tensor_copy(oiT1[:], tp1[:, 0:BLK])
        nc.scalar.copy(oiT2[:], tp2[0:64, 0:BLK])
        state[("oiT", b)] = (oiT1, oiT2)

    def attn_b(b):
        """final attn matmuls + residual add (DVE) + LN2 stats."""
        oiT1, oiT2 = state.pop(("oiT", b))
        xr = state.pop(("xres", b))
        outb = fout.tile([P, 4, D], BF16, tag="outb")
        # all residual adds first so the down-proj psum ring drains promptly
        for t4 in range(4):
            for nh in range(2):
                pf = ps_dn.tile([P, 384], F32, tag="ps_dn", name=f"pf{b}_{t4}_{nh}")
                nc.tensor.matmul(
                    pf[:],
                    oiT1[:, t4 * P : (t4 + 1) * P],
                    wf_s[:, 0, nh * 384 : (nh + 1) * 384],
                    start=True,
                    stop=False,
                )
                nc.tensor.matmul(
                    pf[:],
                    oiT2[:, t4 * P : (t4 + 1) * P],
                    wf_s[:64, 1, nh * 384 : (nh + 1) * 384],
                    start=False,
                    stop=True,
                )
                nc.vector.tensor_add(
                    outb[:, t4, nh * 384 : (nh + 1) * 384],
                    pf[:],
                    xr[:, t4, nh * 384 : (nh + 1) * 384],
                )
        for t4 in range(4):
            t_glob = b * 4 + t4
            stats = fstat.tile([P, 3, 6], F32, tag="bn_stats")
            for sg in range(3):
                nc.vector.bn_stats(
                    stats[:, sg, :], outb[:, t4, sg * 256 : (sg + 1) * 256]
                )
            mv = fstat.tile([P, 2], F32, tag="bn_mv")
            nc.vector.bn_aggr(mv[:], stats[:])
            nc.vector.tensor_copy(mu2[:, t_glob : t_glob + 1], mv[:, 0:1])
            nc.vector.tensor_copy(rstd2[:, t_glob : t_glob + 1], mv[:, 1:2])
        # rstd for this block's 4 tiles
        nc.scalar.activation(
            rstd2[:, b * 4 : b * 4 + 4],
            rstd2[:, b * 4 : b * 4 + 4],
            AF.Sqrt,
            bias=eps_ln[:],
        )
        nc.vector.reciprocal(rstd2[:, b * 4 : b * 4 + 4], rstd2[:, b * 4 : b * 4 + 4])
        state[("out", b)] = outb

    def attn_c(b):
        """LN2 apply + transpose -> hT2 fp8 chunks 0:2 + bf16 chunks 2:6."""
        outb = state[("out", b)]
        hT2f = fhT2.tile([P, N_F8_UP, BLK], F8E4, tag="hT2f", name=f"hT2f_{b}")
        hT2 = fhT2.tile([P, N_DC - N_F8_UP, BLK], BF16, tag="hT2",
                        name=f"hT2_{b}")
        for tt in range(4):
            t_glob = b * 4 + tt
            h2 = fh2.tile([P, D], BF16, tag="h2", name=f"h2_{b}_{tt}")
            nc.vector.tensor_scalar(
                out=h2[:],
                in0=outb[:, tt, :],
                scalar1=mu2[:, t_glob : t_glob + 1],
                scalar2=rstd2[:, t_glob : t_glob + 1],
                op0=ALU.subtract,
                op1=ALU.mult,
            )
            pt = ps_tr.tile([P, D], BF16, tag="ps_tr", name=f"pt3_{b}_{tt}")
            for c in range(N_DC):
                nc.tensor.transpose(
                    pt[:, c * P : (c + 1) * P],
                    h2[:, c * P : (c + 1) * P],
                    identity_b[:],
                )
            nc.vector.tensor_copy(
                hT2f[:, :, tt * P : (tt + 1) * P],
                pt[:, 0 : N_F8_UP * P].rearrange("p (c n) -> p c n", c=N_F8_UP),
            )
            nc.scalar.copy(
                hT2[:, 0:2, tt * P : (tt + 1) * P],
                pt[:, N_F8_UP * P : 4 * P].rearrange("p (c n) -> p c n", c=2),
            )
            nc.scalar.copy(
                hT2[:, 2:4, tt * P : (tt + 1) * P],
                pt[:, 4 * P : D].rearrange("p (c n) -> p c n", c=2),
            )
        state[("hT2", b)] = (hT2f, hT2)

    def mlp_up(b, j_lo, j_hi):
        if j_lo == 0:
            state[("g", b)] = (
                fg.tile([P, N_F8_DN, BLK], F8E4, tag="gf8", name=f"gf8_{b}"),
                fg.tile([P, N_HC - N_F8_DN, BLK], BF16, tag="gb",
                        name=f"gb_{b}"),
            )
        gf8, gb = state[("g", b)]
        hT2f, hT2 = state[("hT2", b)]
        for j in range(j_lo, j_hi):
            pu = ps_up.tile([P, BLK], F32, tag="ps_up")
            nc.tensor.matmul(
                pu[:],
                w1f8_s[:, :, j * P : (j + 1) * P],
                hT2f[:],
                start=True,
                stop=False,
                perf_mode=DR,
            )
            for c in range(N_DC - N_F8_UP):
                nc.tensor.matmul(
                    pu[:],
                    w1_s[:, c, j * P : (j + 1) * P],
                    hT2[:, c, :],
                    start=False,
                    stop=(c == N_DC - N_F8_UP - 1),
                )
            if j < N_F8_DN:
                nc.scalar.activation(
                    gf8[:, j, :], pu[:], AF.Gelu, scale=1.0 / W1_SCALE
                )
            else:
                nc.scalar.activation(
                    gb[:, j - N_F8_DN, :], pu[:], AF.Gelu, scale=1.0 / W1_SCALE
                )
        if j_hi == N_HC:
            state.pop(("hT2", b))

    def mlp_dn(b, tt_lo, tt_hi):
        gf8, gb = state[("g", b)]
        outb = state[("out", b)]
        finb = ffin.tile([P, 2, D], F32, tag="finb", name=f"finb{b}_{tt_lo}")
        for tt in range(tt_lo, tt_hi):
            for nh in range(2):
                py = ps_dn.tile([P, 384], F32, tag="ps_dn", name=f"py{b}_{tt}_{nh}")
                for m in range(N_F8_DN // 2):
                    nc.tensor.matmul(
                        py[:],
                        gf8[:, 2 * m : 2 * m + 2, tt * P : (tt + 1) * P],
                        w2f8_s[:, 2 * m : 2 * m + 2, nh * 384 : (nh + 1) * 384],
                        start=(m == 0),
                        stop=False,
                        perf_mode=DR,
                    )
                for j in range(N_HC - N_F8_DN):
                    nc.tensor.matmul(
                        py[:],
                        gb[:, j, tt * P : (tt + 1) * P],
                        w2_s[:, j, nh * 384 : (nh + 1) * 384],
                        start=False,
                        stop=(j == N_HC - N_F8_DN - 1),
                    )
                nc.vector.tensor_add(
                    finb[:, tt - tt_lo, nh * 384 : (nh + 1) * 384],
                    py[:],
                    outb[:, tt, nh * 384 : (nh + 1) * 384],
                )
            if b == N_BLK - 1:
                # split the final writes so the kernel-end drain is short
                nc.sync.dma_start(
                    y_d.ap()[b * BLK : (b + 1) * BLK, :].rearrange(
                        "(p t) d -> p t d", p=P
                    )[:, tt : tt + 1, :],
                    finb[:, tt - tt_lo : tt - tt_lo + 1, :],
                )
        if b != N_BLK - 1:
            nc.sync.dma_start(
                y_d.ap()[b * BLK : (b + 1) * BLK, :].rearrange(
                    "(p t) d -> p t d", p=P
                )[:, tt_lo:tt_hi, :],
                finb[:],
            )
        if tt_hi == 4:
            state.pop(("g", b))
            state.pop(("out", b))

    # prologue: full attention chain for block 0
    load_xres(0)
    load_xres(1)
    attn_a(0)
    attn_t(0)
    attn_b(0)
    attn_c(0)
    for b in range(N_BLK):
        if b + 2 < N_BLK:
            load_xres(b + 2)
        if b + 1 < N_BLK:
            attn_a(b + 1)
        mlp_up(b, 0, 12)
        if b + 1 < N_BLK:
            attn_t(b + 1)
        mlp_up(b, 12, N_HC)
        if b + 1 < N_BLK:
            attn_b(b + 1)
        mlp_dn(b, 0, 2)
        if b + 1 < N_BLK:
            attn_c(b + 1)
        mlp_dn(b, 2, 4)


def build_nc():
    nc = bacc.Bacc(trn_type="TRN2")

    # Per-core inputs (weights replicated across cores, x sliced per core).
    # Weights arrive pre-tiled host-side into their exact SBUF layouts so
    # every weight DMA is a contiguous per-partition copy (large packets).
    x_d = nc.dram_tensor("x", [S, D], F32R, kind="ExternalInput")
    qw_d = nc.dram_tensor("qw", [P, N_DC, 256], BF16, kind="ExternalInput")
    kw_d = nc.dram_tensor("kw", [P, N_DC, I], BF16, kind="ExternalInput")
    wp_d = nc.dram_tensor("wp", [P, 2, I], BF16, kind="ExternalInput")
    wf_d = nc.dram_tensor("wf", [P, 2, D], BF16, kind="ExternalInput")
    w1f8_d = nc.dram_tensor("w1f8", [P, N_F8_UP, H], F8E4, kind="ExternalInput")
    w1_d = nc.dram_tensor("w1", [P, N_DC - N_F8_UP, H], BF16, kind="ExternalInput")
    w2f8_d = nc.dram_tensor("w2f8", [P, N_F8_DN, D], F8E4, kind="ExternalInput")
    w2_d = nc.dram_tensor("w2", [P, N_HC - N_F8_DN, D], BF16, kind="ExternalInput")
    y_d = nc.dram_tensor("y", [S, D], F32, kind="ExternalOutput")

    with tile.TileContext(nc) as tc:
        with (
            tc.tile_pool(name="const", bufs=1) as const_pool,
            tc.tile_pool(name="p3w", bufs=1) as p3w,
            tc.tile_pool(name="persist", bufs=1) as pers,
        ):
            identity_f = const_pool.tile([P, P], F32)
            make_identity(nc, identity_f[:])
            identity_b = const_pool.tile([P, P], BF16)
            nc.vector.tensor_copy(identity_b[:], identity_f[:])
            ones_f = const_pool.tile([P, 2], F32)
            nc.vector.memset(ones_f[:], 1.0)
            ones_col = const_pool.tile([P, 1], F32R)
            nc.vector.tensor_copy(ones_col[:], ones_f[:, 0:1])
            ones_two = const_pool.tile([1, 2], F32R)
            nc.vector.tensor_copy(ones_two[:], ones_f[0:1, :])
            eps_ln = const_pool.tile([P, 1], F32)
            nc.vector.memset(eps_ln[:], EPS_LN)
            zeros_f = const_pool.tile([P, N_TOK_TILES], F32)
            nc.vector.memset(zeros_f[:], 0.0)
            const = (identity_b, ones_col, ones_two, eps_ln, zeros_f)

            # Persistent per-sample state (phase 1 -> fused phase).
            qA_store = pers.tile([P, N_TOK_TILES, 194], BF16)
            kT1_store = pers.tile([P, N_BLK, BLK], BF16)
            kT2_store = pers.tile([64, N_BLK, BLK], BF16)
            ssk_cols = pers.tile([P, N_TOK_TILES], F32)
            rnk = pers.tile([P, N_TOK_TILES], F32)
            G_row = pers.tile([1, 194], F32R)
            rstdA = pers.tile([1, 2], F32R)
            Gcol1 = pers.tile([P, 1], F32)
            Gcol2 = pers.tile([64, 1], F32)
            wp1_s = pers.tile([P, I], BF16)
            wp2_s = pers.tile([64, I], BF16)
            wf_s = pers.tile([P, 2, D], BF16)
            nc.sync.dma_start(wf_s[:], wf_d.ap())
            mu2 = pers.tile([P, N_TOK_TILES], F32)
            rstd2 = pers.tile([P, N_TOK_TILES], F32)
            persist = (qA_store, kT1_store, kT2_store, ssk_cols, rnk, G_row,
                       rstdA, Gcol1, Gcol2, wp1_s, wp2_s, wf_s,
                       mu2, rstd2)

            w1f8_s = p3w.tile([P, N_F8_UP, H], F8E4)
            w1_s = p3w.tile([P, N_DC - N_F8_UP, H], BF16)
            w2f8_s = p3w.tile([P, N_F8_DN, D], F8E4)
            w2_s = p3w.tile([P, N_HC - N_F8_DN, D], BF16)
            mlpw = (w1f8_s, w1_s, w2f8_s, w2_s)

            _phase1(nc, tc, const,
                    (x_d, qw_d, kw_d, wp_d, w1f8_d, w1_d, w2f8_d, w2_d),
                    persist, mlpw)

            with (
                tc.tile_pool(name="xres", bufs=2) as xb_pool,
                tc.tile_pool(name="foi", bufs=4) as foi,
                tc.tile_pool(name="foiT", bufs=2) as foiT,
                tc.tile_pool(name="fout", bufs=2) as fout,
                tc.tile_pool(name="fh2", bufs=2) as fh2,
                tc.tile_pool(name="fhT2", bufs=2) as fhT2,
                tc.tile_pool(name="fg", bufs=1) as fg,
                tc.tile_pool(name="ffin", bufs=2) as ffin,
                tc.tile_pool(name="fstat", bufs=2) as fstat,
                tc.tile_pool(name="ps_up", bufs=2, space="PSUM") as ps_up,
                tc.tile_pool(name="ps_dn", bufs=2, space="PSUM") as ps_dn,
                tc.tile_pool(name="ps_tr2", bufs=2, space="PSUM") as ps_tr2,
                tc.tile_pool(name="ps_oi", bufs=2, space="PSUM") as ps_oi,
            ):
                pools = (xb_pool, foi, foiT, fout, fh2, fhT2, fg, ffin, fstat,
                         ps_up, ps_dn, ps_tr2, ps_oi)
                _fused_phase(nc, tc, const, (x_d, y_d), persist, mlpw, pools)

    nc.finalize()
    return nc


_NC_CACHE = {}


def _get_nc():
    if "nc" not in _NC_CACHE:
        _NC_CACHE["nc"] = build_nc()
    return _NC_CACHE["nc"]


def kernel(
    x,
    ln1_g,
    ln1_b,
    wq,
    bq,
    wk,
    bk,
    w_g,
    w_proj,
    b_proj,
    w_final,
    b_final,
    ln2_g,
    ln2_b,
    w1,
    b1,
    w2,
    b2,
    _trace=False,
    _trace_kwargs=None,
):
    import ml_dtypes

    x = np.asarray(x, dtype=np.float32)
    f = lambda a: np.asarray(a, dtype=np.float32)
    ln1_g, ln1_b, ln2_g, ln2_b = f(ln1_g), f(ln1_b), f(ln2_g), f(ln2_b)
    wq, bq, wk, bk = f(wq), f(bq), f(wk), f(bk)
    w_g, w_proj, b_proj = f(w_g), f(w_proj), f(b_proj)
    w_final, b_final, w1, b1, w2, b2 = f(w_final), f(b_final), f(w1), f(b1), f(w2), f(b2)

    # The kernel folds LN gains into the weights and relies on all additive
    # biases being zero (guaranteed by the problem's setup_inputs).
    for name, bias in [
        ("ln1_b", ln1_b),
        ("bq", bq),
        ("bk", bk),
        ("b_proj", b_proj),
        ("b_final", b_final),
        ("ln2_b", ln2_b),
        ("b1", b1),
        ("b2", b2),
    ]:
        assert not np.any(bias), f"kernel assumes {name} == 0"

    wq_eff = ln1_g[:, None] * wq  # [768, 192]
    wk_eff = ln1_g[:, None] * wk
    wq_g = wq_eff @ w_g  # [768, 1]
    qw_host = np.concatenate(
        [wq_eff, wq_g, np.zeros((D, 63), np.float32)], axis=1
    ).astype(np.float32)
    w1_eff = (ln2_g[:, None] * w1).astype(ml_dtypes.bfloat16)

    bf = ml_dtypes.bfloat16

    def tile_rows(a, n_chunks):  # [n*128, m] -> [128, n, m]
        return np.ascontiguousarray(
            a.reshape(n_chunks, P, a.shape[1]).transpose(1, 0, 2)
        )

    def split192(a):  # [192, m] -> [128, 2, m] (second slot half-filled)
        out = np.zeros((P, 2, a.shape[1]), dtype=a.dtype)
        out[:, 0, :] = a[0:P]
        out[0:64, 1, :] = a[P:I]
        return out

    f8 = ml_dtypes.float8_e4m3
    w1_scaled = (ln2_g[:, None] * w1) * W1_SCALE  # f32
    cut_up = N_F8_UP * P
    cut_dn = N_F8_DN * P
    nc = _get_nc()
    weights = {
        "qw": tile_rows(qw_host.astype(bf), N_DC),
        "kw": tile_rows(wk_eff.astype(bf), N_DC),
        "wp": split192(w_proj.astype(bf)),
        "wf": split192(w_final.astype(bf)),
        "w1f8": tile_rows(w1_scaled[:cut_up].astype(f8), N_F8_UP),
        "w1": tile_rows(w1_scaled[cut_up:].astype(bf), N_DC - N_F8_UP),
        "w2f8": tile_rows(w2[:cut_dn].astype(f8), N_F8_DN),
        "w2": tile_rows(w2[cut_dn:].astype(bf), N_HC - N_F8_DN),
    }
    in_maps = [dict(weights, x=np.ascontiguousarray(x[i])) for i in range(B)]
    # The first execution after a fresh NEFF load occasionally trips a
    # transient NRT_EXEC_UNIT_UNRECOVERABLE; a retry has always succeeded.
    last_err = None
    for attempt in range(3):
        try:
            res = run_bass_kernel_spmd(
                nc,
                in_maps,
                core_ids=list(range(B)),
                trace=_trace,
                **(_trace_kwargs or {}),
            )
            break
        except Exception as e:  # noqa: BLE001
            last_err = e
            if attempt == 2:
                raise
    else:
        raise last_err
    out = np.stack([res.results[i]["y"] for i in range(B)], axis=0)
    if _trace:
        return out, res
    return out


if __name__ == "__main__":
    print("building...")
    nc = _get_nc()
    print("built")
